# revision 1
# baseline (speedup 1.0000x reference)
"""Deformable transformer decoder layer on 8 NeuronCores.

Sharding: data-parallel over batch (16 batches -> 2 per core). Each core runs
the full decoder layer on its 2 batches; outputs are concatenated.

This is the v1 implementation: jax pmap data-parallel execution across the 8
cores (each core computes its shard independently; no collectives needed).
"""

import numpy as np

DIM, DIM_FF, NH, NL, NP = 256, 1024, 8, 4, 4
HD = DIM // NH
SHAPES = [(64, 64), (32, 32), (16, 16), (8, 8)]
STARTS = [0, 4096, 5120, 5376]
LS = 5440
B, LQ = 16, 1024
LN_EPS = 1e-5
NCORES = 8

_PMAPPED = None


def _build_pmapped():
    import jax
    import jax.numpy as jnp

    def _ln(x, p):
        m = x.mean(-1, keepdims=True)
        v = jnp.var(x, axis=-1, keepdims=True)
        return (x - m) * jax.lax.rsqrt(v + LN_EPS) * p["w"] + p["b"]

    def _mha(q, k, v, p_in, p_out):
        b, l, _ = q.shape
        wq, wk, wv = jnp.split(p_in["w"], 3, 0)
        bq, bk, bv = jnp.split(p_in["b"], 3)
        qh = (q @ wq.T + bq).reshape(b, l, NH, HD)
        kh = (k @ wk.T + bk).reshape(b, l, NH, HD)
        vh = (v @ wv.T + bv).reshape(b, l, NH, HD)
        att = jnp.einsum("bqhd,bkhd->bhqk", qh, kh) / np.sqrt(HD).astype(np.float32)
        att = jax.nn.softmax(att, -1)
        o = jnp.einsum("bhqk,bkhd->bqhd", att, vh).reshape(b, l, DIM)
        return o @ p_out["w"].T + p_out["b"]

    def _msdeform(query, ref, source, source_mask, p):
        b, lq, _ = query.shape
        value = source @ p["val"]["w"].T + p["val"]["b"]
        value = jnp.where(source_mask[..., None], 0.0, value).reshape(b, LS, NH, HD)
        off = (query @ p["off"]["w"].T + p["off"]["b"]).reshape(b, lq, NH, NL, NP, 2)
        aw = (query @ p["aw"]["w"].T + p["aw"]["b"]).reshape(b, lq, NH, NL * NP)
        aw = jax.nn.softmax(aw, -1).reshape(b, lq, NH, NL, NP)
        norm = jnp.asarray([[w, h] for h, w in SHAPES], jnp.float32)
        loc = ref[:, :, None, :, None, :] + off / norm[None, None, None, :, None, :]
        out = jnp.zeros((b, NH, lq, HD), query.dtype)
        for l, (h, w) in enumerate(SHAPES):
            val_l = value[:, STARTS[l]:STARTS[l] + h * w].transpose(0, 2, 1, 3)
            loc_l = loc[:, :, :, l].transpose(0, 2, 1, 3, 4)
            px = loc_l[..., 0] * w - 0.5
            py = loc_l[..., 1] * h - 0.5
            x0f = jnp.floor(px)
            y0f = jnp.floor(py)
            fx = px - x0f
            fy = py - y0f
            x0 = x0f.astype(jnp.int32)
            y0 = y0f.astype(jnp.int32)

            def corner(xi, yi):
                valid = (xi >= 0) & (xi < w) & (yi >= 0) & (yi < h)
                idx = jnp.clip(yi, 0, h - 1) * w + jnp.clip(xi, 0, w - 1)
                g = jnp.take_along_axis(val_l, idx.reshape(b, NH, lq * NP, 1), axis=2)
                return g.reshape(b, NH, lq, NP, HD) * valid[..., None]

            samp = (corner(x0, y0) * ((1 - fx) * (1 - fy))[..., None]
                    + corner(x0 + 1, y0) * (fx * (1 - fy))[..., None]
                    + corner(x0, y0 + 1) * ((1 - fx) * fy)[..., None]
                    + corner(x0 + 1, y0 + 1) * (fx * fy)[..., None])
            aw_l = aw[:, :, :, l].transpose(0, 2, 1, 3)
            out = out + (samp * aw_l[..., None]).sum(3)
        out = out.transpose(0, 2, 1, 3).reshape(b, lq, DIM)
        return out @ p["out"]["w"].T + p["out"]["b"], loc

    def layer(input, pos, reference_point, source, source_mask, params):
        x = input
        h = _ln(x, params["self_norm"])
        q = h + pos
        x = x + _mha(q, q, h, params["mha_in"], params["mha_out"])
        h = _ln(x, params["cross_norm"])
        ca, loc = _msdeform(h + pos, reference_point, source, source_mask, params)
        x = x + ca
        h = _ln(x, params["ff_norm"])
        ff = jax.nn.relu(h @ params["ff1"]["w"].T + params["ff1"]["b"]) \
            @ params["ff2"]["w"].T + params["ff2"]["b"]
        return x + ff, loc

    devs = jax.devices()[:NCORES]
    return jax.pmap(layer, devices=devs, static_broadcasted_argnums=()), jax, devs


def kernel(input, pos, reference_point, source, source_shape, level_start,
           source_mask, params):
    global _PMAPPED
    if _PMAPPED is None:
        _PMAPPED = _build_pmapped()
    pmapped, jax, devs = _PMAPPED

    per = B // NCORES
    shard = lambda a: np.ascontiguousarray(np.asarray(a)).reshape(
        (NCORES, per) + tuple(np.asarray(a).shape[1:]))
    params_rep = jax.tree.map(
        lambda a: np.broadcast_to(np.asarray(a), (NCORES,) + tuple(np.asarray(a).shape)),
        params)
    out, loc = pmapped(shard(input), shard(pos), shard(reference_point),
                       shard(source), shard(source_mask), params_rep)
    out = np.asarray(out).reshape(B, LQ, DIM)
    loc = np.asarray(loc).reshape(B, LQ, NH, NL, NP, 2)
    return out, loc
